# revision 38
# baseline (speedup 1.0000x reference)
"""MiniMaxText01 MoE layer on 8 Trainium2 NeuronCores.

Expert-parallel sparse routing: core e owns expert e's weights and processes
only the tokens whose top-2 routing includes expert e (~1062 of 4096, padded
to CAP=1152).  The host performs the dispatch (gathers each expert's tokens,
the "all-to-all" of the sharding hint) and the combine (scatter-add of the
two weighted expert outputs per token).  All model math runs on device:
  router (bf16 logits -> exp -> top-2 denominator -> own-expert weight)
  -> SwiGLU MLP in bf16 with fp32 PSUM accumulation -> per-token scaling.
This is 4x fewer matmul columns and 8x less weight DMA per core than the
dense token-parallel version.

Everything runs in a "transposed" layout: tokens on the matmul free dim,
features on partitions, so both MLP matmuls need no on-device transposes,
with the weight tiles pre-arranged on the host.
"""

import numpy as np
import ml_dtypes

import concourse.bass as bass
import concourse.mybir as mybir
import concourse.tile as tile
from concourse.bass_utils import run_bass_kernel_spmd
from concourse.masks import make_identity

# ---------------------------------------------------------------------------
# Workaround: this walrus build rejects instructions carrying >1 sem wait on
# the TileContext kernel-tail Drain ("Too many sync wait commands").  Split
# the accumulated waits so each SP instruction carries at most one.
from concourse.tile import TileContext
from concourse.vector_clock import ScopedClock


def _drain_and_barrier_split(self, tick_clock, wait_clock):
    drain_inst = self.nc.sync.drain()
    wait_clock.add_sem_waits(
        drain_inst.ins, ScopedClock({None: tick_clock.global_clock})
    )
    inst = drain_inst.ins
    waits = list(inst.sync_info.on_wait)
    if len(waits) > 1:
        inst.sync_info.on_wait = [waits[0]]
        for w in waits[1:]:
            nop = self.nc.sync.nop()
            nop.ins.sync_info = mybir.SyncInfo(on_wait=[w], on_update=[])
    self.nc.all_engine_barrier()
    assert self.sems is not None
    popped = self.nc._tile_sem_poison_stack.pop()
    assert popped is self._sem_poison
    self.nc.clear_and_free_semaphores(list(self.sems.allocated().values()))
    self.nc.all_engine_barrier()


TileContext._drain_and_barrier = _drain_and_barrier_split


def _split_sync_waits(nc, maxw=1):
    """Same workaround, applied to every instruction: move excess sem-waits
    onto freshly inserted same-engine NOPs placed just before the
    instruction (per-engine program order is preserved)."""
    import bass_rust

    ctr = 0
    for f in nc.m.functions:
        for bb in f.blocks:
            out = []
            changed = False
            for inst in bb.instructions:
                si = inst.sync_info
                waits = list(si.on_wait) if si is not None else []
                if len(waits) > maxw:
                    for w in waits[:-maxw]:
                        ctr += 1
                        out.append(
                            bass_rust.InstNoOp(
                                name=f"I-wsplit-{ctr}",
                                engine=inst.engine,
                                ins=[],
                                outs=[],
                                sync_info=mybir.SyncInfo(
                                    on_wait=[w], on_update=[]
                                ),
                            )
                        )
                    si.on_wait = waits[-maxw:]
                    changed = True
                out.append(inst)
            if changed:
                bb.instructions = out
# ---------------------------------------------------------------------------

def _elide_ldweights(nc):
    """The tile lowering emits one InstLdweights per matmul.  With the
    k-outer/chunk-inner ordering, consecutive matmuls reuse the same
    stationary tile, so the repeat loads are redundant: the PE array
    already holds the weights.  Replace each repeat with a same-engine
    NOP carrying identical sync_info (waits/updates preserved exactly).
    Conservatively resets tracking on transposes (they load the PE array)
    and on any non-matmul PE instruction."""
    import bass_rust

    def key_of(ap):
        return (
            str(getattr(ap, "memref", None)),
            getattr(ap, "offset", None),
            tuple(tuple(x) for x in ap.ap),
            str(ap.dtype),
        )

    removed = 0
    for f in nc.m.functions:
        for bb in f.blocks:
            out = []
            last_key = None
            for inst in bb.instructions:
                if str(inst.engine) != "EngineType.PE":
                    out.append(inst)
                    continue
                tn = type(inst).__name__
                if tn == "InstLdweights":
                    k = key_of(inst.ins[0])
                    if k == last_key:
                        removed += 1
                        si = inst.sync_info
                        if si is not None and (si.on_wait or si.on_update):
                            out.append(
                                bass_rust.InstNoOp(
                                    name=f"{inst.name}-elided",
                                    engine=inst.engine,
                                    ins=[],
                                    outs=[],
                                    sync_info=si,
                                )
                            )
                        continue
                    last_key = k
                elif tn == "InstMatmult":
                    if inst.is_transpose:
                        last_key = None
                else:
                    last_key = None
                out.append(inst)
            bb.instructions = out
    return removed


BF16 = ml_dtypes.bfloat16

E, TOPK, H, I = 8, 2, 2048, 5632
T = 4096
NCORES = 8
P = 128
KH = H // P               # 16  H-chunks
MI = 2 * I // P           # 88  2I-tiles (g: 0..43, u: 44..87)
NI = I // P               # 44  I-tiles
HT = H // P               # 16  H-tiles

def _plan_cap(max_count):
    """Token capacity + free-dim chunking (PSUM bank limit: 512 fp32 wide).
    The capacity is split into ceil(cap/512) NEAR-EQUAL chunks rather than
    512-wide ones plus a skinny tail: total matmul columns are identical,
    but every matmul then has enough moving columns to hide the stationary
    weight load (a 40-wide tail matmul cannot).  The router works on
    128-token tiles so it pads to xf_cap = ceil(cap/128)*128."""
    cap = ((max_count + 1) // 2) * 2
    n = (cap + 511) // 512
    base = cap // n
    chunks = []
    off = 0
    for j in range(n):
        w = ((base + 3) // 4) * 4
        if j == n - 1:
            w = cap - off
        chunks.append((off, w))
        off += w
    assert off == cap and all(w <= 512 for _, w in chunks), chunks
    xf_cap = ((cap + P - 1) // P) * P
    return cap, chunks, xf_cap


_CACHE = {}


def _build_kernel(CAP, CHUNKS, XF_CAP, sim_safe=False, route_at=4):
    """sim_safe: CoreSim lacks Silu; emulate it as g*sigmoid(g) (extra mult)."""
    CT = XF_CAP // P
    nc = bass.Bass()
    f32 = mybir.dt.float32
    bf16 = mybir.dt.bfloat16

    # gathered tokens, feature-major (for MLP matmuls)
    xt_d = nc.dram_tensor("xt", [P, KH * CAP], bf16, kind="ExternalInput")
    # gathered tokens, token-tile-major (router lhsT)
    xf_d = nc.dram_tensor("xf", [CT, P, KH * P], bf16, kind="ExternalInput")
    # gate weights^T with an extra 9th column = this core's own expert row
    gwx_d = nc.dram_tensor("gwx", [P, KH * (E + 1)], bf16, kind="ExternalInput")
    w13_d = nc.dram_tensor("w13", [MI, P, KH * P], bf16, kind="ExternalInput")
    w2_d = nc.dram_tensor("w2", [HT, P, NI * P], bf16, kind="ExternalInput")
    out_d = nc.dram_tensor("out", [P, HT * CAP], f32, kind="ExternalOutput")

    Act = mybir.ActivationFunctionType
    Alu = mybir.AluOpType

    ROUTE_AT = route_at   # run the router after this many g/u i-tiles

    with tile.TileContext(nc) as tc:
        with (
            tc.tile_pool(name="const", bufs=1) as const_pool,
            tc.tile_pool(name="resident", bufs=1) as res_pool,
        ):
            # resident tensors (xt DMA'd per token-chunk so the first g/u
            # matmul group can start as soon as its slab + first weights land)
            xt_sb = res_pool.tile([P, KH, CAP], bf16)
            gwx_sb = const_pool.tile([P, KH, E + 1], bf16)
            ht_sb = res_pool.tile([P, NI, CAP], bf16)
            wbc_sb = res_pool.tile([P, CAP], f32)   # own-expert token weights
            ones_sb = const_pool.tile([1, P], f32)
            nc.vector.memset(ones_sb[:], 1.0)
            ident_sb = const_pool.tile([P, P], f32)
            make_identity(nc, ident_sb)

            def router(rt_pool, rtp_pool, dwT_sb):
                # per 128-token tile: logits[tok, 0:8] for the top-2
                # denominator, logits[tok, 8] = own expert.
                # weight = exp(l8-mx) / (m1+m2).
                for tt in range(CT):
                    xf_sb = rt_pool.tile([P, KH, P], bf16, name="xf")
                    nc.sync.dma_start(
                        xf_sb[:], xf_d[tt].rearrange("p (k t) -> p k t", k=KH)
                    )
                    # router PSUM tiles all share one bank-sized slot name so
                    # the pool fits in 2 banks next to the 6-bank gu pool
                    psum_l = rtp_pool.tile([P, 512], f32, name="r")[:, 0 : E + 1]
                    for k in range(KH):
                        nc.tensor.matmul(
                            psum_l[:],
                            xf_sb[:, k, :],
                            gwx_sb[:, k, :],
                            start=(k == 0),
                            stop=(k == KH - 1),
                        )
                    mx = rt_pool.tile([P, 1], f32, name="mx")
                    nc.vector.reduce_max(
                        mx[:], psum_l[:, 0:E], axis=mybir.AxisListType.X
                    )
                    nmx = rt_pool.tile([P, 1], f32, name="nmx")
                    nc.vector.tensor_scalar_mul(nmx[:], mx[:], -1.0)
                    p_sb = rt_pool.tile([P, E + 1], f32, name="p")
                    nc.scalar.activation(p_sb[:], psum_l[:], Act.Exp, bias=nmx[:])
                    m1 = rt_pool.tile([P, 1], f32, name="m1")
                    nc.vector.reduce_max(
                        m1[:], p_sb[:, 0:E], axis=mybir.AxisListType.X
                    )
                    # pm = p where p < m1 else 0  (knock out the top-1)
                    pm = rt_pool.tile([P, E], f32, name="pm")
                    nc.vector.scalar_tensor_tensor(
                        pm[:], p_sb[:, 0:E], m1[:], p_sb[:, 0:E],
                        Alu.is_lt, Alu.mult,
                    )
                    m2 = rt_pool.tile([P, 1], f32, name="m2")
                    nc.vector.reduce_max(m2[:], pm[:], axis=mybir.AxisListType.X)
                    denom = rt_pool.tile([P, 1], f32, name="den")
                    nc.vector.tensor_add(denom[:], m1[:], m2[:])
                    rden = rt_pool.tile([P, 1], f32, name="rden")
                    nc.vector.reciprocal(rden[:], denom[:])
                    # own-expert weight (host already decided membership)
                    ow = rt_pool.tile([P, 1], f32, name="ow")
                    nc.vector.tensor_scalar_mul(ow[:], p_sb[:, E : E + 1], rden[:])
                    psum_t = rtp_pool.tile([P, 512], f32, name="r")[0:1, 0:P]
                    nc.tensor.transpose(psum_t, ow[:], ident_sb[:])
                    nc.vector.tensor_copy(
                        dwT_sb[0:1, tt * P : (tt + 1) * P], psum_t
                    )
                # broadcast own token weights across partitions
                for off, w in CHUNKS:
                    psum_b = rtp_pool.tile([P, 512], f32, name="r")[:, 0:w]
                    nc.tensor.matmul(
                        psum_b, ones_sb[:], dwT_sb[0:1, off : off + w],
                        start=True, stop=True,
                    )
                    nc.scalar.copy(wbc_sb[:, off : off + w], psum_b)

            # ---------------- expert MLP ----------------
            with (
                tc.tile_pool(name="w13p", bufs=6) as w13_pool,
                tc.tile_pool(name="w2p", bufs=2) as w2_pool,
                tc.tile_pool(name="tmp", bufs=4) as tmp_pool,
                tc.tile_pool(name="outp", bufs=3) as out_pool,
            ):
                w2_tiles = [None] * HT

                def fetch_w2(h):
                    w2t = w2_pool.tile([P, NI, P], bf16, name="w2")
                    nc.sync.dma_start(
                        w2t[:], w2_d[h].rearrange("p (i m) -> p i m", i=NI)
                    )
                    w2_tiles[h] = w2t
                w13_tiles = [None] * NI

                def fetch(i):
                    wg = w13_pool.tile([P, KH, P], bf16, name="w13")
                    nc.sync.dma_start(
                        wg[:], w13_d[i].rearrange("p (k m) -> p k m", k=KH)
                    )
                    wu = w13_pool.tile([P, KH, P], bf16, name="w13")
                    nc.sync.dma_start(
                        wu[:], w13_d[NI + i].rearrange("p (k m) -> p k m", k=KH)
                    )
                    w13_tiles[i] = (wg, wu)

                # DMA order: the first weight pair, then the xt chunk-slabs
                # (all on the i=0 critical path), then everything else --
                # the second pair is only needed one iteration later.
                fetch(0)
                xt_r = xt_d[:].rearrange("p (k t) -> p k t", k=KH)
                for off, w in CHUNKS:
                    nc.sync.dma_start(
                        xt_sb[:, :, off : off + w], xt_r[:, :, off : off + w]
                    )
                fetch(1)
                nc.sync.dma_start(
                    gwx_sb[:], gwx_d[:].rearrange("p (k e) -> p k e", k=KH)
                )
                fetch_w2(0)

                with tc.tile_pool(name="gup", bufs=6, space="PSUM") as gu_psum:
                    for i in range(NI):
                        if i == ROUTE_AT:
                            # router rides here: its xf DMAs + PE work
                            # overlap the weight stream / g-u compute
                            with (
                                tc.tile_pool(name="rt", bufs=2) as rt_pool,
                                tc.tile_pool(
                                    name="rtp", bufs=2, space="PSUM"
                                ) as rtp_pool,
                                tc.tile_pool(name="dw", bufs=1) as dw_pool,
                            ):
                                dwT_sb = dw_pool.tile([1, XF_CAP], f32)
                                router(rt_pool, rtp_pool, dwT_sb)
                        if i + 2 < NI:
                            fetch(i + 2)
                        wg, wu = w13_tiles[i]
                        w13_tiles[i] = None
                        # k-outer / chunk-inner: the 3 chunk matmuls of each
                        # weight tile are consecutive, so after LdWeights
                        # elision the PE loads each stationary tile once per
                        # k (3 chunks of compute per load).
                        psg = [gu_psum.tile([P, w], f32, name="gu")
                               for _, w in CHUNKS]
                        psu = [gu_psum.tile([P, w], f32, name="gu")
                               for _, w in CHUNKS]
                        for k in range(KH):
                            for c, (off, w) in enumerate(CHUNKS):
                                nc.tensor.matmul(
                                    psg[c][:],
                                    wg[:, k, :],
                                    xt_sb[:, k, off : off + w],
                                    start=(k == 0),
                                    stop=(k == KH - 1),
                                )
                        for k in range(KH):
                            for c, (off, w) in enumerate(CHUNKS):
                                nc.tensor.matmul(
                                    psu[c][:],
                                    wu[:, k, :],
                                    xt_sb[:, k, off : off + w],
                                    start=(k == 0),
                                    stop=(k == KH - 1),
                                )
                        for c, (off, w) in enumerate(CHUNKS):
                            sg = tmp_pool.tile([P, w], f32, name="sg")
                            if sim_safe:
                                nc.scalar.activation(
                                    sg[:], psg[c][:], Act.Sigmoid
                                )
                                nc.vector.tensor_tensor(
                                    sg[:], sg[:], psg[c][:], Alu.mult
                                )
                            else:
                                nc.scalar.activation(sg[:], psg[c][:], Act.Silu)
                            nc.vector.tensor_tensor(
                                ht_sb[:, i, off : off + w], sg[:], psu[c][:],
                                Alu.mult,
                            )

                # ---------------- down projection ----------------
                with tc.tile_pool(name="yp", bufs=6, space="PSUM") as y_psum:
                    for h in range(HT):
                        if h + 1 < HT:
                            fetch_w2(h + 1)
                        w2t = w2_tiles[h]
                        w2_tiles[h] = None
                        psy = [y_psum.tile([P, w], f32, name="py")
                               for _, w in CHUNKS]
                        for i in range(NI):
                            for c, (off, w) in enumerate(CHUNKS):
                                nc.tensor.matmul(
                                    psy[c][:],
                                    w2t[:, i, :],
                                    ht_sb[:, i, off : off + w],
                                    start=(i == 0),
                                    stop=(i == NI - 1),
                                )
                        for c, (off, w) in enumerate(CHUNKS):
                            ty = out_pool.tile([P, w], f32, name="ty")
                            nc.vector.tensor_tensor(
                                ty[:], psy[c][:], wbc_sb[:, off : off + w],
                                Alu.mult,
                            )
                            nc.sync.dma_start(
                                out_d[:, h * CAP + off : h * CAP + off + w],
                                ty[:],
                            )

    return nc


def _route(hidden_states, gate_w):
    """Replicate the reference's fp32 router on host to decide membership
    (the dispatch); the weight *values* are recomputed on device.  Softmax
    is monotone per row, so top-k on logits = top-k on probs; two argmax
    passes tie-break by lowest index, exactly like jax.lax.top_k."""
    x = np.asarray(hidden_states, np.float32)
    gw = np.asarray(gate_w, np.float32)
    logits = x @ gw.T
    i1 = np.argmax(logits, axis=-1)
    masked = logits.copy()
    masked[np.arange(T), i1] = -np.inf
    i2 = np.argmax(masked, axis=-1)
    return np.stack([i1, i2], axis=1)


def _prep_inputs(hidden_states, gate_w, w13, w2):
    x = np.asarray(hidden_states, np.float32)
    gate_w = np.asarray(gate_w, np.float32)

    idx = _route(x, gate_w)                      # [T, 2]
    member = np.zeros((T, E), bool)
    member[np.arange(T)[:, None], idx] = True
    ids_list = [np.nonzero(member[:, e])[0] for e in range(E)]
    counts = [len(ids) for ids in ids_list]
    cap, chunks, xf_cap = _plan_cap(max(counts))

    x_pad = np.concatenate([x, np.zeros((1, H), np.float32)], 0)  # row T = 0

    # weight tiles (shared layout, sliced per expert)
    w13b = np.asarray(w13).astype(BF16)
    w13d = w13b.reshape(E, MI, P, KH, P).transpose(0, 1, 4, 3, 2)
    w2b = np.asarray(w2).astype(BF16)
    w2d = w2b.reshape(E, HT, P, NI, P).transpose(0, 1, 4, 3, 2)

    gwT = gate_w.reshape(E, KH, P).transpose(2, 1, 0)  # [P, KH, E]

    ct = xf_cap // P
    in_maps = []
    for e in range(E):
        ids_pad = np.full(xf_cap, T, np.int64)
        ids_pad[: counts[e]] = ids_list[e]
        xg = x_pad[ids_pad].astype(BF16)         # [xf_cap, H]
        xt = np.ascontiguousarray(
            xg[:cap].reshape(cap, KH, P).transpose(2, 1, 0).reshape(P, KH * cap)
        )
        xf = np.ascontiguousarray(
            xg.reshape(ct, P, KH, P).transpose(0, 3, 2, 1).reshape(ct, P, KH * P)
        )
        gwx = np.concatenate([gwT, gwT[:, :, e : e + 1]], axis=2)  # [P,KH,9]
        gwx = np.ascontiguousarray(gwx.reshape(P, KH * (E + 1)).astype(BF16))
        in_maps.append(
            {
                "xt": xt,
                "xf": xf,
                "gwx": gwx,
                "w13": np.ascontiguousarray(w13d[e].reshape(MI, P, KH * P)),
                "w2": np.ascontiguousarray(w2d[e].reshape(HT, P, NI * P)),
            }
        )
    return in_maps, ids_list, counts, (cap, chunks, xf_cap)


def kernel(hidden_states, gate_w, w13, w2, top_k):
    import time

    assert int(top_k) == TOPK
    t0 = time.time()
    in_maps, ids_list, counts, plan = _prep_inputs(hidden_states, gate_w, w13, w2)
    cap, chunks, xf_cap = plan
    t1 = time.time()
    if _CACHE.get("plan") != plan:
        nc = _build_kernel(cap, chunks, xf_cap)
        nel = _elide_ldweights(nc)
        print(f"[kernel] elided {nel} redundant ldweights", flush=True)
        _split_sync_waits(nc)
        _CACHE["nc"] = nc
        _CACHE["plan"] = plan
    nc = _CACHE["nc"]
    t2 = time.time()
    res = run_bass_kernel_spmd(nc, in_maps, core_ids=list(range(NCORES)))
    t3 = time.time()
    print(
        f"[kernel] prep {t1 - t0:.1f}s  build {t2 - t1:.1f}s  run {t3 - t2:.1f}s"
        f"  cap {cap}",
        flush=True,
    )
    _CACHE["last_results"] = res

    # combine: scatter-add each expert's weighted outputs
    out = np.zeros((T, H), np.float32)
    for e in range(E):
        oc = res.results[e]["out"]               # [128, 16*cap]
        ye = oc.reshape(P, HT, cap).transpose(2, 1, 0).reshape(cap, H)
        out[ids_list[e]] += ye[: counts[e]]
    return out


# revision 39
# speedup vs baseline: 1.0089x; 1.0089x over previous
"""MiniMaxText01 MoE layer on 8 Trainium2 NeuronCores.

Expert-parallel sparse routing: core e owns expert e's weights and processes
only the tokens whose top-2 routing includes expert e (~1062 of 4096, padded
to CAP=1152).  The host performs the dispatch (gathers each expert's tokens,
the "all-to-all" of the sharding hint) and the combine (scatter-add of the
two weighted expert outputs per token).  All model math runs on device:
  router (bf16 logits -> exp -> top-2 denominator -> own-expert weight)
  -> SwiGLU MLP in bf16 with fp32 PSUM accumulation -> per-token scaling.
This is 4x fewer matmul columns and 8x less weight DMA per core than the
dense token-parallel version.

Everything runs in a "transposed" layout: tokens on the matmul free dim,
features on partitions, so both MLP matmuls need no on-device transposes,
with the weight tiles pre-arranged on the host.
"""

import numpy as np
import ml_dtypes

import concourse.bass as bass
import concourse.mybir as mybir
import concourse.tile as tile
from concourse.bass_utils import run_bass_kernel_spmd
from concourse.masks import make_identity

# ---------------------------------------------------------------------------
# Workaround: this walrus build rejects instructions carrying >1 sem wait on
# the TileContext kernel-tail Drain ("Too many sync wait commands").  Split
# the accumulated waits so each SP instruction carries at most one.
from concourse.tile import TileContext
from concourse.vector_clock import ScopedClock


def _drain_and_barrier_split(self, tick_clock, wait_clock):
    drain_inst = self.nc.sync.drain()
    wait_clock.add_sem_waits(
        drain_inst.ins, ScopedClock({None: tick_clock.global_clock})
    )
    inst = drain_inst.ins
    waits = list(inst.sync_info.on_wait)
    if len(waits) > 1:
        inst.sync_info.on_wait = [waits[0]]
        for w in waits[1:]:
            nop = self.nc.sync.nop()
            nop.ins.sync_info = mybir.SyncInfo(on_wait=[w], on_update=[])
    self.nc.all_engine_barrier()
    assert self.sems is not None
    popped = self.nc._tile_sem_poison_stack.pop()
    assert popped is self._sem_poison
    self.nc.clear_and_free_semaphores(list(self.sems.allocated().values()))
    self.nc.all_engine_barrier()


TileContext._drain_and_barrier = _drain_and_barrier_split


def _split_sync_waits(nc, maxw=1):
    """Same workaround, applied to every instruction: move excess sem-waits
    onto freshly inserted same-engine NOPs placed just before the
    instruction (per-engine program order is preserved)."""
    import bass_rust

    ctr = 0
    for f in nc.m.functions:
        for bb in f.blocks:
            out = []
            changed = False
            for inst in bb.instructions:
                si = inst.sync_info
                waits = list(si.on_wait) if si is not None else []
                if len(waits) > maxw:
                    for w in waits[:-maxw]:
                        ctr += 1
                        out.append(
                            bass_rust.InstNoOp(
                                name=f"I-wsplit-{ctr}",
                                engine=inst.engine,
                                ins=[],
                                outs=[],
                                sync_info=mybir.SyncInfo(
                                    on_wait=[w], on_update=[]
                                ),
                            )
                        )
                    si.on_wait = waits[-maxw:]
                    changed = True
                out.append(inst)
            if changed:
                bb.instructions = out
# ---------------------------------------------------------------------------

def _elide_ldweights(nc):
    """The tile lowering emits one InstLdweights per matmul.  With the
    k-outer/chunk-inner ordering, consecutive matmuls reuse the same
    stationary tile, so the repeat loads are redundant: the PE array
    already holds the weights.  Replace each repeat with a same-engine
    NOP carrying identical sync_info (waits/updates preserved exactly).
    Conservatively resets tracking on transposes (they load the PE array)
    and on any non-matmul PE instruction."""
    import bass_rust

    def key_of(ap):
        return (
            str(getattr(ap, "memref", None)),
            getattr(ap, "offset", None),
            tuple(tuple(x) for x in ap.ap),
            str(ap.dtype),
        )

    removed = 0
    for f in nc.m.functions:
        for bb in f.blocks:
            out = []
            last_key = None
            for inst in bb.instructions:
                if str(inst.engine) != "EngineType.PE":
                    out.append(inst)
                    continue
                tn = type(inst).__name__
                if tn == "InstLdweights":
                    k = key_of(inst.ins[0])
                    if k == last_key:
                        removed += 1
                        si = inst.sync_info
                        if si is not None and (si.on_wait or si.on_update):
                            out.append(
                                bass_rust.InstNoOp(
                                    name=f"{inst.name}-elided",
                                    engine=inst.engine,
                                    ins=[],
                                    outs=[],
                                    sync_info=si,
                                )
                            )
                        continue
                    last_key = k
                elif tn == "InstMatmult":
                    if inst.is_transpose:
                        last_key = None
                else:
                    last_key = None
                out.append(inst)
            bb.instructions = out
    return removed


BF16 = ml_dtypes.bfloat16

E, TOPK, H, I = 8, 2, 2048, 5632
T = 4096
NCORES = 8
P = 128
KH = H // P               # 16  H-chunks
MI = 2 * I // P           # 88  2I-tiles (g: 0..43, u: 44..87)
NI = I // P               # 44  I-tiles
HT = H // P               # 16  H-tiles

def _plan_cap(max_count):
    """Token capacity + free-dim chunking (PSUM bank limit: 512 fp32 wide).
    The capacity is split into ceil(cap/512) NEAR-EQUAL chunks rather than
    512-wide ones plus a skinny tail: total matmul columns are identical,
    but every matmul then has enough moving columns to hide the stationary
    weight load (a 40-wide tail matmul cannot).  The router works on
    128-token tiles so it pads to xf_cap = ceil(cap/128)*128."""
    cap = ((max_count + 1) // 2) * 2
    n = (cap + 511) // 512
    base = cap // n
    chunks = []
    off = 0
    for j in range(n):
        w = ((base + 3) // 4) * 4
        if j == n - 1:
            w = cap - off
        chunks.append((off, w))
        off += w
    assert off == cap and all(w <= 512 for _, w in chunks), chunks
    xf_cap = ((cap + P - 1) // P) * P
    return cap, chunks, xf_cap


_CACHE = {}


def _build_kernel(CAP, CHUNKS, XF_CAP, sim_safe=False, route_at=4):
    """sim_safe: CoreSim lacks Silu; emulate it as g*sigmoid(g) (extra mult)."""
    CT = XF_CAP // P
    nc = bass.Bass()
    f32 = mybir.dt.float32
    bf16 = mybir.dt.bfloat16

    # gathered tokens, feature-major (for MLP matmuls)
    xt_d = nc.dram_tensor("xt", [P, KH * CAP], bf16, kind="ExternalInput")
    # gathered tokens, token-tile-major (router lhsT)
    xf_d = nc.dram_tensor("xf", [CT, P, KH * P], bf16, kind="ExternalInput")
    # gate weights^T with an extra 9th column = this core's own expert row
    gwx_d = nc.dram_tensor("gwx", [P, KH * (E + 1)], bf16, kind="ExternalInput")
    w13_d = nc.dram_tensor("w13", [MI, P, KH * P], bf16, kind="ExternalInput")
    w2_d = nc.dram_tensor("w2", [HT, P, NI * P], bf16, kind="ExternalInput")
    out_d = nc.dram_tensor("out", [P, HT * CAP], f32, kind="ExternalOutput")

    Act = mybir.ActivationFunctionType
    Alu = mybir.AluOpType

    ROUTE_AT = route_at   # run the router after this many g/u i-tiles

    with tile.TileContext(nc) as tc:
        with (
            tc.tile_pool(name="const", bufs=1) as const_pool,
            tc.tile_pool(name="resident", bufs=1) as res_pool,
        ):
            # resident tensors (xt DMA'd per token-chunk so the first g/u
            # matmul group can start as soon as its slab + first weights land)
            xt_sb = res_pool.tile([P, KH, CAP], bf16)
            gwx_sb = const_pool.tile([P, KH, E + 1], bf16)
            ht_sb = res_pool.tile([P, NI, CAP], bf16)
            wbc_sb = res_pool.tile([P, CAP], f32)   # own-expert token weights
            ones_sb = const_pool.tile([1, P], f32)
            nc.vector.memset(ones_sb[:], 1.0)
            ident_sb = const_pool.tile([P, P], f32)
            make_identity(nc, ident_sb)

            def router(rt_pool, rtp_pool, dwT_sb):
                # per 128-token tile: logits[tok, 0:8] for the top-2
                # denominator, logits[tok, 8] = own expert.
                # weight = exp(l8-mx) / (m1+m2).
                for tt in range(CT):
                    xf_sb = rt_pool.tile([P, KH, P], bf16, name="xf")
                    nc.sync.dma_start(
                        xf_sb[:], xf_d[tt].rearrange("p (k t) -> p k t", k=KH)
                    )
                    # router PSUM tiles all share one bank-sized slot name so
                    # the pool fits in 2 banks next to the 6-bank gu pool
                    psum_l = rtp_pool.tile([P, 512], f32, name="r")[:, 0 : E + 1]
                    for k in range(KH):
                        nc.tensor.matmul(
                            psum_l[:],
                            xf_sb[:, k, :],
                            gwx_sb[:, k, :],
                            start=(k == 0),
                            stop=(k == KH - 1),
                        )
                    mx = rt_pool.tile([P, 1], f32, name="mx")
                    nc.vector.reduce_max(
                        mx[:], psum_l[:, 0:E], axis=mybir.AxisListType.X
                    )
                    nmx = rt_pool.tile([P, 1], f32, name="nmx")
                    nc.vector.tensor_scalar_mul(nmx[:], mx[:], -1.0)
                    p_sb = rt_pool.tile([P, E + 1], f32, name="p")
                    nc.scalar.activation(p_sb[:], psum_l[:], Act.Exp, bias=nmx[:])
                    m1 = rt_pool.tile([P, 1], f32, name="m1")
                    nc.vector.reduce_max(
                        m1[:], p_sb[:, 0:E], axis=mybir.AxisListType.X
                    )
                    # pm = p where p < m1 else 0  (knock out the top-1)
                    pm = rt_pool.tile([P, E], f32, name="pm")
                    nc.vector.scalar_tensor_tensor(
                        pm[:], p_sb[:, 0:E], m1[:], p_sb[:, 0:E],
                        Alu.is_lt, Alu.mult,
                    )
                    m2 = rt_pool.tile([P, 1], f32, name="m2")
                    nc.vector.reduce_max(m2[:], pm[:], axis=mybir.AxisListType.X)
                    denom = rt_pool.tile([P, 1], f32, name="den")
                    nc.vector.tensor_add(denom[:], m1[:], m2[:])
                    rden = rt_pool.tile([P, 1], f32, name="rden")
                    nc.vector.reciprocal(rden[:], denom[:])
                    # own-expert weight (host already decided membership)
                    ow = rt_pool.tile([P, 1], f32, name="ow")
                    nc.vector.tensor_scalar_mul(ow[:], p_sb[:, E : E + 1], rden[:])
                    psum_t = rtp_pool.tile([P, 512], f32, name="r")[0:1, 0:P]
                    nc.tensor.transpose(psum_t, ow[:], ident_sb[:])
                    nc.vector.tensor_copy(
                        dwT_sb[0:1, tt * P : (tt + 1) * P], psum_t
                    )
                # broadcast own token weights across partitions
                for off, w in CHUNKS:
                    psum_b = rtp_pool.tile([P, 512], f32, name="r")[:, 0:w]
                    nc.tensor.matmul(
                        psum_b, ones_sb[:], dwT_sb[0:1, off : off + w],
                        start=True, stop=True,
                    )
                    nc.scalar.copy(wbc_sb[:, off : off + w], psum_b)

            # ---------------- expert MLP ----------------
            with (
                tc.tile_pool(name="w13p", bufs=6) as w13_pool,
                tc.tile_pool(name="w2p", bufs=2) as w2_pool,
                tc.tile_pool(name="tmp", bufs=4) as tmp_pool,
                tc.tile_pool(name="outp", bufs=3) as out_pool,
            ):
                w2_tiles = [None] * HT

                def fetch_w2(h):
                    w2t = w2_pool.tile([P, NI, P], bf16, name="w2")
                    nc.sync.dma_start(
                        w2t[:], w2_d[h].rearrange("p (i m) -> p i m", i=NI)
                    )
                    w2_tiles[h] = w2t
                w13_tiles = [None] * NI

                def fetch(i):
                    wg = w13_pool.tile([P, KH, P], bf16, name="w13")
                    nc.sync.dma_start(
                        wg[:], w13_d[i].rearrange("p (k m) -> p k m", k=KH)
                    )
                    wu = w13_pool.tile([P, KH, P], bf16, name="w13")
                    nc.sync.dma_start(
                        wu[:], w13_d[NI + i].rearrange("p (k m) -> p k m", k=KH)
                    )
                    w13_tiles[i] = (wg, wu)

                # DMA order: the first weight pair, then the xt chunk-slabs
                # (all on the i=0 critical path), then everything else --
                # the second pair is only needed one iteration later.
                fetch(0)
                xt_r = xt_d[:].rearrange("p (k t) -> p k t", k=KH)
                for k in range(KH):
                    nc.sync.dma_start(
                        xt_sb[:, k : k + 1, :], xt_r[:, k : k + 1, :]
                    )
                fetch(1)
                nc.sync.dma_start(
                    gwx_sb[:], gwx_d[:].rearrange("p (k e) -> p k e", k=KH)
                )
                fetch_w2(0)

                with tc.tile_pool(name="gup", bufs=6, space="PSUM") as gu_psum:
                    for i in range(NI):
                        if i == ROUTE_AT:
                            # router rides here: its xf DMAs + PE work
                            # overlap the weight stream / g-u compute
                            with (
                                tc.tile_pool(name="rt", bufs=2) as rt_pool,
                                tc.tile_pool(
                                    name="rtp", bufs=2, space="PSUM"
                                ) as rtp_pool,
                                tc.tile_pool(name="dw", bufs=1) as dw_pool,
                            ):
                                dwT_sb = dw_pool.tile([1, XF_CAP], f32)
                                router(rt_pool, rtp_pool, dwT_sb)
                        if i + 2 < NI:
                            fetch(i + 2)
                        wg, wu = w13_tiles[i]
                        w13_tiles[i] = None
                        # k-outer / chunk-inner: the 3 chunk matmuls of each
                        # weight tile are consecutive, so after LdWeights
                        # elision the PE loads each stationary tile once per
                        # k (3 chunks of compute per load).
                        psg = [gu_psum.tile([P, w], f32, name="gu")
                               for _, w in CHUNKS]
                        psu = [gu_psum.tile([P, w], f32, name="gu")
                               for _, w in CHUNKS]
                        for k in range(KH):
                            for c, (off, w) in enumerate(CHUNKS):
                                nc.tensor.matmul(
                                    psg[c][:],
                                    wg[:, k, :],
                                    xt_sb[:, k, off : off + w],
                                    start=(k == 0),
                                    stop=(k == KH - 1),
                                )
                        for k in range(KH):
                            for c, (off, w) in enumerate(CHUNKS):
                                nc.tensor.matmul(
                                    psu[c][:],
                                    wu[:, k, :],
                                    xt_sb[:, k, off : off + w],
                                    start=(k == 0),
                                    stop=(k == KH - 1),
                                )
                        for c, (off, w) in enumerate(CHUNKS):
                            sg = tmp_pool.tile([P, w], f32, name="sg")
                            if sim_safe:
                                nc.scalar.activation(
                                    sg[:], psg[c][:], Act.Sigmoid
                                )
                                nc.vector.tensor_tensor(
                                    sg[:], sg[:], psg[c][:], Alu.mult
                                )
                            else:
                                nc.scalar.activation(sg[:], psg[c][:], Act.Silu)
                            nc.vector.tensor_tensor(
                                ht_sb[:, i, off : off + w], sg[:], psu[c][:],
                                Alu.mult,
                            )

                # ---------------- down projection ----------------
                with tc.tile_pool(name="yp", bufs=6, space="PSUM") as y_psum:
                    for h in range(HT):
                        if h + 1 < HT:
                            fetch_w2(h + 1)
                        w2t = w2_tiles[h]
                        w2_tiles[h] = None
                        psy = [y_psum.tile([P, w], f32, name="py")
                               for _, w in CHUNKS]
                        for i in range(NI):
                            for c, (off, w) in enumerate(CHUNKS):
                                nc.tensor.matmul(
                                    psy[c][:],
                                    w2t[:, i, :],
                                    ht_sb[:, i, off : off + w],
                                    start=(i == 0),
                                    stop=(i == NI - 1),
                                )
                        for c, (off, w) in enumerate(CHUNKS):
                            ty = out_pool.tile([P, w], f32, name="ty")
                            nc.vector.tensor_tensor(
                                ty[:], psy[c][:], wbc_sb[:, off : off + w],
                                Alu.mult,
                            )
                            nc.sync.dma_start(
                                out_d[:, h * CAP + off : h * CAP + off + w],
                                ty[:],
                            )

    return nc


def _route(hidden_states, gate_w):
    """Replicate the reference's fp32 router on host to decide membership
    (the dispatch); the weight *values* are recomputed on device.  Softmax
    is monotone per row, so top-k on logits = top-k on probs; two argmax
    passes tie-break by lowest index, exactly like jax.lax.top_k."""
    x = np.asarray(hidden_states, np.float32)
    gw = np.asarray(gate_w, np.float32)
    logits = x @ gw.T
    i1 = np.argmax(logits, axis=-1)
    masked = logits.copy()
    masked[np.arange(T), i1] = -np.inf
    i2 = np.argmax(masked, axis=-1)
    return np.stack([i1, i2], axis=1)


def _prep_inputs(hidden_states, gate_w, w13, w2):
    x = np.asarray(hidden_states, np.float32)
    gate_w = np.asarray(gate_w, np.float32)

    idx = _route(x, gate_w)                      # [T, 2]
    member = np.zeros((T, E), bool)
    member[np.arange(T)[:, None], idx] = True
    ids_list = [np.nonzero(member[:, e])[0] for e in range(E)]
    counts = [len(ids) for ids in ids_list]
    cap, chunks, xf_cap = _plan_cap(max(counts))

    x_pad = np.concatenate([x, np.zeros((1, H), np.float32)], 0)  # row T = 0

    # weight tiles (shared layout, sliced per expert)
    w13b = np.asarray(w13).astype(BF16)
    w13d = w13b.reshape(E, MI, P, KH, P).transpose(0, 1, 4, 3, 2)
    w2b = np.asarray(w2).astype(BF16)
    w2d = w2b.reshape(E, HT, P, NI, P).transpose(0, 1, 4, 3, 2)

    gwT = gate_w.reshape(E, KH, P).transpose(2, 1, 0)  # [P, KH, E]

    ct = xf_cap // P
    in_maps = []
    for e in range(E):
        ids_pad = np.full(xf_cap, T, np.int64)
        ids_pad[: counts[e]] = ids_list[e]
        xg = x_pad[ids_pad].astype(BF16)         # [xf_cap, H]
        xt = np.ascontiguousarray(
            xg[:cap].reshape(cap, KH, P).transpose(2, 1, 0).reshape(P, KH * cap)
        )
        xf = np.ascontiguousarray(
            xg.reshape(ct, P, KH, P).transpose(0, 3, 2, 1).reshape(ct, P, KH * P)
        )
        gwx = np.concatenate([gwT, gwT[:, :, e : e + 1]], axis=2)  # [P,KH,9]
        gwx = np.ascontiguousarray(gwx.reshape(P, KH * (E + 1)).astype(BF16))
        in_maps.append(
            {
                "xt": xt,
                "xf": xf,
                "gwx": gwx,
                "w13": np.ascontiguousarray(w13d[e].reshape(MI, P, KH * P)),
                "w2": np.ascontiguousarray(w2d[e].reshape(HT, P, NI * P)),
            }
        )
    return in_maps, ids_list, counts, (cap, chunks, xf_cap)


def kernel(hidden_states, gate_w, w13, w2, top_k):
    import time

    assert int(top_k) == TOPK
    t0 = time.time()
    in_maps, ids_list, counts, plan = _prep_inputs(hidden_states, gate_w, w13, w2)
    cap, chunks, xf_cap = plan
    t1 = time.time()
    if _CACHE.get("plan") != plan:
        nc = _build_kernel(cap, chunks, xf_cap)
        nel = _elide_ldweights(nc)
        print(f"[kernel] elided {nel} redundant ldweights", flush=True)
        _split_sync_waits(nc)
        _CACHE["nc"] = nc
        _CACHE["plan"] = plan
    nc = _CACHE["nc"]
    t2 = time.time()
    res = run_bass_kernel_spmd(nc, in_maps, core_ids=list(range(NCORES)))
    t3 = time.time()
    print(
        f"[kernel] prep {t1 - t0:.1f}s  build {t2 - t1:.1f}s  run {t3 - t2:.1f}s"
        f"  cap {cap}",
        flush=True,
    )
    _CACHE["last_results"] = res

    # combine: scatter-add each expert's weighted outputs
    out = np.zeros((T, H), np.float32)
    for e in range(E):
        oc = res.results[e]["out"]               # [128, 16*cap]
        ye = oc.reshape(P, HT, cap).transpose(2, 1, 0).reshape(cap, H)
        out[ids_list[e]] += ye[: counts[e]]
    return out
